# revision 31
# baseline (speedup 1.0000x reference)
"""GCN encoder (3x GCNConv + residual + final linear) on 8 trn2 NeuronCores.

V3 strategy (aggregate-first, transpose-free, chunked early AllGathers):
  * Same degree-quantized node layout as V2: nodes sorted by in-degree;
    rank r -> bin (r % 392), position j (r // 392).  Per position j the
    slot budget s_j = max over bins of deg, so every (core, block) shares
    one compile-time constant slot->dst selection matrix (bf16, SBUF).
  * AGGREGATE-FIRST: since S(hW) = (Sh)W, the table holds RAW h*dinv
    (bf16) rather than (hW)*dinv.  The pre-collective phase per block is
    just one fused scale+cast (ACT) + one DMA, so the AllGather starts
    almost immediately after a block's epilogue.
  * TRANSPOSE-FREE: the segment-sum matmul is emitted as
    acc[feat,dst] += G_chunk^T @ sconst_chunk  (lhsT=G, rhs=sconst), and
    the self-loop term rides the same PSUM accumulation as
    hcast_blk^T @ I (a transpose-by-identity matmul).  The layer weight
    then applies as z0[dst,fout] = matmul(lhsT=acc_copy, rhs=W) with NO
    explicit transposes anywhere in the layer.  Epilogue:
    hn = relu(z0*dinv_dst + b) + h_prev  (dinv_dst is a per-partition
    scalar after the W matmul).
  * The AllGather is split into 4 chunks (blocks [0:25), [25:37),
    [37:45), [45:49)); chunk k for layer l+1 is issued as soon as the
    last block of chunk k finishes its epilogue during layer l, so all
    but the small final chunk overlap the running gather phase.
  * Gather: one dma_gather per block (49 calls round-robin over 4 SWDGE
    queues).  Trailing pad slots (beyond the 2059 real slots) use a
    negative index so the HW drops them (sconst is zero there, so the
    garbage rows never contribute).  Interior pads still fetch the
    guaranteed-zero pad row.
Pad nodes have dinv=0 so their table rows stay exactly zero in every layer.
"""

import os
import numpy as np
import ml_dtypes

import concourse.bass as bass
import concourse.mybir as mybir
import concourse.tile as tile
import concourse.bacc as bacc
from concourse.bass_utils import run_bass_kernel_spmd
from concourse.masks import make_identity

N = 50000
E = 800000
D_IN = 128
D_H = 128
D_OUT = 64
NCORES = 8
P = 128
PER_CORE = 6272          # 49*128
NBLK = PER_CORE // P     # 49
NBINS = NCORES * NBLK    # 392
ROWS = NCORES * PER_CORE # table rows (50176)
HALF = ROWS // 2         # 25088: gather view base row
# AllGather chunk boundaries (blocks); overridable for experiments
CHUNK_BLOCKS = tuple(
    int(v) for v in os.environ.get("GCN_CB", "0,25,37,45,49").split(","))
# each chunk is issued as AGSPLIT row-slice collectives at one dep point
# (concurrent CCOM channels can overlap them); the table layout follows
# the flat sub-boundary list
AGSPLIT = int(os.environ.get("GCN_AGSPLIT", "1"))


def _flat_bounds():
    cb = CHUNK_BLOCKS
    out = [0]
    for k in range(len(cb) - 1):
        n = cb[k + 1] - cb[k]
        ns = max(1, min(AGSPLIT, n))
        step, rem = divmod(n, ns)
        b = cb[k]
        for i in range(ns):
            b += step + (1 if i < rem else 0)
            out.append(b)
    return tuple(out)


FLAT_BLOCKS = _flat_bounds()

LAST_EXEC_NS = None
LAST_RESULTS = None
LAST_NC = None
LAST_IN_MAPS = None
LAST_PLAN = None


def _preprocess(x, edge_index):
    src = edge_index[0].astype(np.int64)
    dst = edge_index[1].astype(np.int64)
    deg = np.bincount(dst, minlength=N)
    dinv = (1.0 / np.sqrt((deg + 1).astype(np.float32))).astype(np.float32)

    # degree-quantized deal: rank r -> bin r % NBINS, position j = r // NBINS
    order = np.argsort(-deg, kind="stable")
    node_bin = np.empty(N, np.int64)
    node_pos = np.empty(N, np.int64)
    r = np.arange(N)
    node_bin[order] = r % NBINS
    node_pos[order] = r // NBINS

    core_of = node_bin % NCORES
    blk_of = node_bin // NCORES
    pos_of = blk_of * P + node_pos

    # chunked table layout: chunk k holds blocks [cb[k], cb[k+1]) of every
    # core, ordered [chunk][core][local row]
    cb = FLAT_BLOCKS
    chunk_of_blk = np.zeros(NBLK, np.int64)
    for k in range(len(cb) - 1):
        chunk_of_blk[cb[k]:cb[k + 1]] = k
    rows_per_core = [(cb[k + 1] - cb[k]) * P for k in range(len(cb) - 1)]
    chunk_base = np.concatenate([[0], np.cumsum([NCORES * n for n in rows_per_core])])[:-1]
    k_of = chunk_of_blk[blk_of]
    trow = (chunk_base[k_of] + core_of * np.array(rows_per_core)[k_of]
            + (pos_of - np.array(cb[:-1])[k_of] * P))

    # uniform slot schedule: s_j = max degree at position j over all bins
    padded_deg = np.zeros(NBINS * P, np.int64)
    padded_deg[:N] = deg[order]
    s = padded_deg.reshape(P, NBINS).max(axis=1)          # [128]
    B = np.zeros(P + 1, np.int64)
    B[1:] = np.cumsum(s)
    slot_raw = int(B[P])
    NCH = (slot_raw + P - 1) // P                          # chunks per block
    SLOT_TOTAL = NCH * P

    # pad row: last row of the table must belong to a pad node (rank >= N)
    # (bin NBINS-1, j P-1) has rank (P-1)*NBINS + NBINS-1 = NBINS*P - 1 >= N
    assert NBINS * P - 1 >= N
    # that node is core 7, blk 48, pos 127 -> last row of last chunk
    PAD_IDX = ROWS - 1 - HALF
    DROP_IDX = -HALF  # trailing slots: dropped by HW (sconst zero there)

    # constant selection matrix [128, NCH*128]
    sconst = np.zeros((P, NCH * P), np.float32)
    for j in range(P):
        for slot in range(int(B[j]), int(B[j] + s[j])):
            sconst[slot % P, (slot // P) * P + j] = 1.0
    sconst16 = np.ascontiguousarray(sconst.astype(ml_dtypes.bfloat16))

    # per-(core, block) gather index lists
    idx_all = np.full((NCORES, NBLK, SLOT_TOTAL), PAD_IDX, np.int32)
    idx_all[:, :, slot_raw:] = DROP_IDX
    # group edges by destination node, sources sorted for locality
    eorder = np.argsort(dst * np.int64(ROWS) + trow[src], kind="stable")
    dst_s = dst[eorder]
    srow_s = trow[src[eorder]]
    cuts = np.searchsorted(dst_s, np.arange(N + 1))
    k_in_dst = np.arange(E) - cuts[dst_s]          # edge rank within its dst
    slot_e = B[node_pos[dst_s]] + k_in_dst
    idx_all[core_of[dst_s], blk_of[dst_s], slot_e] = srow_s - HALF

    # one gather call per block (the last block is split across all 4
    # queues to balance the 49 = 4*12+1 round-robin) -> wrapped int16 idx
    ncols = NBLK * SLOT_TOTAL // 16
    gidx = np.zeros((NCORES, P, ncols), np.int16)
    gmeta = []
    cc = 0
    for b in range(NBLK):
        if b < NBLK - 1:
            calls = [(0, NCH)]
        else:
            q = NCH // 4
            calls = [(i * q, (i + 1) * q) for i in range(3)] + [(3 * q, NCH)]
        for qi, (c0, c1) in enumerate(calls):
            gmeta.append(dict(b=b, c0=c0, c1=c1, ni=(c1 - c0) * P,
                              col0=cc + c0 * P // 16,
                              queue=(b % 4) if b < NBLK - 1 else qi))
        for c in range(NCORES):
            flat = idx_all[c, b].reshape(-1).astype(np.int16)
            w = flat.reshape(-1, 16).T
            gidx[c, :, cc:cc + SLOT_TOTAL // 16] = np.tile(w, (8, 1))
        cc += SLOT_TOTAL // 16

    # x/dinv shards + output mapping
    x_sh = np.zeros((NCORES, PER_CORE, D_IN), np.float32)
    dinv_sh = np.zeros((NCORES, P, NBLK), np.float32)
    node_of = np.full((NCORES, PER_CORE), -1, np.int64)
    node_of[core_of, pos_of] = np.arange(N)
    for c in range(NCORES):
        valid = node_of[c] >= 0
        x_sh[c, valid] = x[node_of[c][valid]]
        dv = np.zeros(PER_CORE, np.float32)
        dv[valid] = dinv[node_of[c][valid]]
        dinv_sh[c] = dv.reshape(NBLK, P).T

    # layer-1 table: the pre-scaled input x*dinv in trow order, replicated
    # to every core (input distribution, so layer 1 needs no AllGather)
    t1 = np.zeros((ROWS + P, D_IN), np.float32)
    t1[trow] = x * dinv[:, None]
    t1 = np.ascontiguousarray(t1.astype(ml_dtypes.bfloat16))

    plan = dict(gmeta=gmeta, ncols=ncols, NCH=NCH, slot_raw=slot_raw)
    return plan, gidx, sconst16, x_sh, dinv_sh, node_of, t1


def _build(plan, reps=None, skip=()):
    if reps is None:
        reps = int(os.environ.get("GCN_REPS", "1"))
    skip = set(skip) | set(filter(None, os.environ.get("GCN_SKIP", "").split(",")))
    f32 = mybir.dt.float32
    bf16 = mybir.dt.bfloat16
    i16 = mybir.dt.int16
    gmeta, ncols, NCH = plan["gmeta"], plan["ncols"], plan["NCH"]
    calls_of = {b: [m for m in gmeta if m["b"] == b] for b in range(NBLK)}
    # slots beyond slot_raw are never gathered (trailing drop); restrict the
    # last chunk's matmul to the real partitions so garbage is never read
    nfull, krem = divmod(plan["slot_raw"], P)
    cb = CHUNK_BLOCKS
    # gather element size in elements (128 = one 256B row; 256 = 512B
    # overlapping two rows, halving nothing but testing the desc-rate regime)
    esz = int(os.environ.get("GCN_ELEM", "128"))

    nc = bacc.Bacc("TRN2", target_bir_lowering=False, debug=False,
                   enable_asserts=True, num_devices=NCORES,
                   num_swdge_queues=4)

    x_t = nc.dram_tensor("x_sh", [PER_CORE, D_IN], f32, kind="ExternalInput")
    t1_t = nc.dram_tensor("t1", [ROWS + P, D_IN], bf16, kind="ExternalInput")
    w_t = [nc.dram_tensor(f"w{i}", [D_H, D_H], f32, kind="ExternalInput") for i in range(3)]
    b_t = [nc.dram_tensor(f"b{i}", [P, D_H], f32, kind="ExternalInput") for i in range(3)]
    wout_t = nc.dram_tensor("wout", [D_H, D_OUT], f32, kind="ExternalInput")
    bout_t = nc.dram_tensor("bout", [P, D_OUT], f32, kind="ExternalInput")
    dinv_t = nc.dram_tensor("dinv_sh", [P, NBLK], f32, kind="ExternalInput")
    idx_t = nc.dram_tensor("gidx", [P, ncols], i16, kind="ExternalInput")
    s_t = nc.dram_tensor("sconst", [P, NCH * P], bf16, kind="ExternalInput")
    y_t = nc.dram_tensor("y_sh", [PER_CORE, D_OUT], f32, kind="ExternalOutput")

    rg = [list(range(NCORES))]
    add = mybir.AluOpType.add
    mult = mybir.AluOpType.mult
    relu = mybir.ActivationFunctionType.Relu
    copyf = mybir.ActivationFunctionType.Copy

    with tile.TileContext(nc) as tc:
        with (
            tc.tile_pool(name="const", bufs=1) as cpool,
            tc.tile_pool(name="work", bufs=3) as wpool,
            tc.tile_pool(name="gbuf", bufs=int(os.environ.get("GCN_GBUFS", "8"))) as gpool,
            tc.tile_pool(name="hbuf", bufs=2) as hpool,
            tc.tile_pool(name="psum", bufs=2, space="PSUM") as ppool,
            tc.tile_pool(name="dram", bufs=2, space="DRAM") as dpool,
        ):
            ident_f = cpool.tile([P, P], f32, name="ident_f")
            make_identity(nc, ident_f[:])
            ident_b = cpool.tile([P, P], bf16, name="ident_b")
            nc.vector.tensor_copy(ident_b[:], ident_f[:])

            wt, bt = [], []
            for i in range(3):
                w_s = cpool.tile([D_H, D_H], f32, name=f"w_s{i}")
                nc.sync.dma_start(out=w_s[:], in_=w_t[i][:])
                b_s = cpool.tile([P, D_H], f32, name=f"b_s{i}")
                nc.sync.dma_start(out=b_s[:], in_=b_t[i][:])
                wt.append(w_s)
                bt.append(b_s)
            wout_s = cpool.tile([D_H, D_OUT], f32)
            nc.sync.dma_start(out=wout_s[:], in_=wout_t[:])
            bout_s = cpool.tile([P, D_OUT], f32)
            nc.sync.dma_start(out=bout_s[:], in_=bout_t[:])
            dinv_s = cpool.tile([P, NBLK], f32)
            nc.sync.dma_start(out=dinv_s[:], in_=dinv_t[:])
            idx_s = cpool.tile([P, ncols], i16)
            nc.sync.dma_start(out=idx_s[:], in_=idx_t[:])
            sconst_s = cpool.tile([P, NCH * P], bf16)
            nc.sync.dma_start(out=sconst_s[:], in_=s_t[:])

            def pre_block(h, hcast, ag_in, b):
                """scale+cast block b of h into hcast and ship to ag_in."""
                hcb = hcast[:, b * P:(b + 1) * P]
                if "pre" in skip:
                    if b == 0:
                        nc.vector.memset(hcast[:], 0.0)
                else:
                    nc.scalar.activation(hcb, h[:, b * P:(b + 1) * P],
                                         copyf, scale=dinv_s[:, b:b + 1])
                if ag_in is not None:
                    nc.sync.dma_start(out=ag_in[b * P:(b + 1) * P, :], in_=hcb)

            hier = os.environ.get("GCN_HIER", "0") == "1"
            rg_die = [[0, 1, 2, 3], [4, 5, 6, 7]]
            rg_x = [[0, 6], [1, 7], [2, 4], [3, 5]]

            fb = FLAT_BLOCKS

            def maybe_ag(table, ag_in, b):
                """issue AllGather chunk k if block b completes it."""
                if "coll" in skip:
                    return
                for k in range(len(cb) - 1):
                    if b + 1 == cb[k + 1]:
                        r0, r1 = cb[k] * P, cb[k + 1] * P
                        n = r1 - r0
                        g0 = NCORES * r0
                        if not hier:
                            # issue every layout sub-unit of this chunk as
                            # its own collective; concurrent CCOM channels
                            # can overlap them
                            for i in range(len(fb) - 1):
                                if not (cb[k] <= fb[i] and fb[i + 1] <= cb[k + 1]):
                                    continue
                                s0, s1 = fb[i] * P, fb[i + 1] * P
                                nc.gpsimd.collective_compute(
                                    "AllGather", mybir.AluOpType.bypass,
                                    replica_groups=rg,
                                    ins=[ag_in[s0:s1, :]],
                                    outs=[table[NCORES * s0:NCORES * s1, :]],
                                )
                        else:
                            maxn = max(cb[j + 1] - cb[j]
                                       for j in range(len(cb) - 1)) * P
                            stage = dpool.tile([4 * maxn, D_H], bf16,
                                               tag="stage", name="stage")
                            # stage a: gather 4 shards within each die
                            nc.gpsimd.collective_compute(
                                "AllGather", mybir.AluOpType.bypass,
                                replica_groups=rg_die,
                                ins=[ag_in[r0:r1, :]],
                                outs=[stage[0:4 * n, :]],
                            )
                            # stage b: exchange die-blocks across the dies
                            nc.gpsimd.collective_compute(
                                "AllGather", mybir.AluOpType.bypass,
                                replica_groups=rg_x,
                                ins=[stage[0:4 * n, :]],
                                outs=[table[g0:g0 + NCORES * n, :]],
                            )

            for rep in range(reps):
                h = hpool.tile([P, NBLK * P], f32, tag="h", name="h0")
                hcast = hpool.tile([P, NBLK * P], bf16, tag="hc", name="hc0")
                shared = os.environ.get("GCN_SHARED", "0") == "1"
                aspace = "Shared" if shared else "Local"
                table = None  # layer 1 gathers straight from the t1 input
                for b in range(NBLK):
                    nc.sync.dma_start(out=h[:, b * P:(b + 1) * P],
                                      in_=x_t[b * P:(b + 1) * P, :])
                    pre_block(h, hcast, None, b)

                for layer in range(3):
                    last = layer == 2
                    gv = t1_t[HALF:, :] if table is None else table[HALF:, :]
                    if esz != P:
                        # overlapping-row view: elem_step=P, elem_size=esz
                        gv = bass.AP(tensor=gv.tensor, offset=gv.offset,
                                     ap=[[P, ROWS - HALF], [1, esz]])
                    gather_view = gv
                    if not last:
                        ag_next = dpool.tile([PER_CORE, D_H], bf16,
                                             tag="ag_in", name=f"ag_in{layer + 1}")
                        table_next = dpool.tile([ROWS + P, D_H], bf16,
                                                tag="table", addr_space=aspace,
                                                name=f"table{layer + 1}")
                        hn = hpool.tile([P, NBLK * P], f32, tag="h",
                                        name=f"h{layer + 1}")
                        hcast_next = hpool.tile([P, NBLK * P], bf16, tag="hc",
                                                name=f"hc{layer + 1}")

                    for b in range(NBLK):
                        G = gpool.tile([P, NCH, esz], bf16, tag="G", name="G")
                        if "gather" not in skip:
                            for m in calls_of[b]:
                                nc.gpsimd.dma_gather(
                                    out_ap=G[:, m["c0"]:m["c1"], :],
                                    in_ap=gather_view,
                                    idxs_ap=idx_s[:, m["col0"]:m["col0"] + m["ni"] // 16],
                                    num_idxs=m["ni"], num_idxs_reg=m["ni"],
                                    elem_size=esz, elem_step=P,
                                    single_packet=False,
                                    queue_num=m["queue"],
                                )
                        else:
                            nc.vector.memset(G[:, 0:1, :], 0.0)

                        acc = ppool.tile([P, P], f32, tag="acc", name="acc")
                        if "mm" in skip:
                            nc.vector.memset(acc[:], 0.0)
                        else:
                            # self-loop: acc = hcast_blk^T (transpose via I)
                            nc.tensor.matmul(acc[:], lhsT=hcast[:, b * P:(b + 1) * P],
                                             rhs=ident_b[:], start=True, stop=False)
                            for c in range(nfull):
                                nc.tensor.matmul(
                                    acc[:], lhsT=G[:, c, 0:P],
                                    rhs=sconst_s[:, c * P:(c + 1) * P],
                                    start=False, stop=(krem == 0 and c == nfull - 1))
                            if krem:
                                nc.tensor.matmul(
                                    acc[:], lhsT=G[0:krem, nfull, 0:P],
                                    rhs=sconst_s[0:krem, nfull * P:(nfull + 1) * P],
                                    start=False, stop=True)

                        if "epi" in skip:
                            if not last and b == 0:
                                nc.vector.memset(hn[:], 0.0)
                            continue
                        # z0[dst, fout] = (agg @ W); agg^T sits in acc
                        a0 = wpool.tile([P, P], f32, tag="a0", name="a0")
                        nc.vector.tensor_copy(a0[:], acc[:])
                        zp = ppool.tile([P, P], f32, tag="zp", name="zp")
                        nc.tensor.matmul(zp[:], lhsT=a0[:], rhs=wt[layer][:],
                                         start=True, stop=True)
                        t = wpool.tile([P, P], f32, tag="t", name="t")
                        nc.vector.tensor_scalar_mul(t[:], zp[:], dinv_s[:, b:b + 1])
                        nc.vector.tensor_tensor(out=t[:], in0=t[:], in1=bt[layer][:],
                                                op=add)
                        if last:
                            # h3 block -> final projection, fused into layer 3
                            h3b = wpool.tile([P, P], f32, tag="h3b", name="h3b")
                            nc.scalar.activation(h3b[:], t[:], relu)
                            nc.vector.tensor_tensor(out=h3b[:], in0=h3b[:],
                                                    in1=h[:, b * P:(b + 1) * P], op=add)
                            tp = ppool.tile([P, P], f32, tag="acc", name="tp")
                            nc.tensor.transpose(tp[:], h3b[:], ident_f[:])
                            hT = wpool.tile([P, P], f32, tag="a0", name="hT")
                            nc.vector.tensor_copy(hT[:], tp[:])
                            yp = ppool.tile([P, D_OUT], f32, tag="zp", name="yp")
                            nc.tensor.matmul(yp[:], lhsT=hT[:], rhs=wout_s[:],
                                             start=True, stop=True)
                            yt = wpool.tile([P, D_OUT], f32, tag="t", name="yt")
                            nc.vector.tensor_tensor(out=yt[:], in0=yp[:],
                                                    in1=bout_s[:], op=add)
                            nc.sync.dma_start(out=y_t[b * P:(b + 1) * P, :], in_=yt[:])
                        else:
                            hnb = hn[:, b * P:(b + 1) * P]
                            nc.scalar.activation(hnb, t[:], relu)
                            nc.vector.tensor_tensor(out=hnb, in0=hnb,
                                                    in1=h[:, b * P:(b + 1) * P], op=add)
                            pre_block(hn, hcast_next, ag_next, b)
                            maybe_ag(table_next, ag_next, b)

                    if not last:
                        h = hn
                        hcast = hcast_next
                        ag_in = ag_next
                        table = table_next

    nc.compile()
    return nc


def _in_maps(inputs, gidx, sconst16, x_sh, dinv_sh, t1):
    W_out = np.asarray(inputs["W_out"], np.float32)
    b_out = np.asarray(inputs["b_out"], np.float32)
    maps = []
    for c in range(NCORES):
        m = {
            "x_sh": np.ascontiguousarray(x_sh[c]),
            "t1": t1,
            "dinv_sh": np.ascontiguousarray(dinv_sh[c]),
            "gidx": np.ascontiguousarray(gidx[c]),
            "sconst": sconst16,
            "wout": W_out,
            "bout": np.ascontiguousarray(np.broadcast_to(b_out[None, :], (P, D_OUT))),
        }
        for i in range(3):
            m[f"w{i}"] = np.asarray(inputs[f"W{i}"], np.float32)
            m[f"b{i}"] = np.ascontiguousarray(
                np.broadcast_to(np.asarray(inputs[f"b{i}"], np.float32)[None, :], (P, D_H)))
        maps.append(m)
    return maps


def kernel(x, edge_index, W0, b0, W1, b1, W2, b2, W_out, b_out):
    global LAST_EXEC_NS, LAST_RESULTS, LAST_NC, LAST_IN_MAPS, LAST_PLAN
    x = np.asarray(x, dtype=np.float32)
    edge_index = np.asarray(edge_index, dtype=np.int32)

    plan, gidx, sconst16, x_sh, dinv_sh, node_of, t1 = _preprocess(x, edge_index)
    nc = _build(plan)

    inputs = dict(W0=W0, b0=b0, W1=W1, b1=b1, W2=W2, b2=b2,
                  W_out=W_out, b_out=b_out)
    in_maps = _in_maps(inputs, gidx, sconst16, x_sh, dinv_sh, t1)

    trace = os.environ.get("GCN_TRACE", "0") == "1"
    LAST_NC = nc
    LAST_IN_MAPS = in_maps
    LAST_PLAN = plan
    res = run_bass_kernel_spmd(nc, in_maps, list(range(NCORES)), trace=trace)
    LAST_EXEC_NS = res.exec_time_ns
    LAST_RESULTS = res

    y = np.empty((N, D_OUT), np.float32)
    for c in range(NCORES):
        valid = node_of[c] >= 0
        y[node_of[c][valid]] = res.results[c]["y_sh"][valid]
    return y
